# revision 26
# baseline (speedup 1.0000x reference)
"""Trainium2 Bass kernel: 16-head RoPE attention block (B=4, T=2048, D=2048).

Sharding: tensor-parallel over heads. Each of the 8 cores owns 2 heads
(a 256-wide slice of the q/k/v projection output features). Host sums
the 8 partial out-projection products (the "all-reduce").

Design (vs the two-pass fp32r baseline):
  - Matmul operands in bf16 (fp32 PSUM accumulate): halves DMA traffic.
  - Per-batch fusion: q/k/v live in SBUF (no DRAM scratch round-trip),
    with interleaved emission s1(b0) s2(b0) s1(b1) s3(b0) s2(b1) ... so
    the PE queue never drains at batch boundaries (no HAM re-throttle).
  - Stage 2 runs two 1024-wide kti sweeps per head: paired 2-bank score
    psums (one 1024-wide exp per pair), k/v stationaries shared across
    the q halves. The softmax denominator is accumulated elementwise
    over e-tiles on the vector engine in fp16 (all-16-bit tensor_tensor
    hits the 2x DVE mode; fp16 keeps the 15-add chain at ~0.2% error),
    then reduced over k-partitions by a rank-1 ones matmul.
  - Normalization stays on-chip: the [1,T] denominator row is
    re-partitioned to [128,16] via an SBUF->SBUF DMA, reciprocal'd on
    the vector engine, scattered back, and broadcast to [128,T] with a
    rank-1 ones matmul; the broadcast+multiply tail is emitted one
    block late so the PE never waits on the reciprocal round-trip.
  - Stage 3 packs two d-tiles per 2-bank psum with one wide evacuation
    and a single 3D-AP DMA per pair; outputs in bf16, host sums fp32.
"""

import math

import numpy as np
import ml_dtypes

import concourse.bacc as bacc
import concourse.bass as bass
import concourse.mybir as mybir
import concourse.tile as tile
from concourse.bass_utils import run_bass_kernel_spmd

F32 = mybir.dt.float32
F32R = mybir.dt.float32r
BF16 = mybir.dt.bfloat16
F16 = mybir.dt.float16
EXP = mybir.ActivationFunctionType.Exp
COPY = mybir.ActivationFunctionType.Copy

# Problem shape (hardcoded; the harness calls kernel() with exactly these).
B = 4
T = 2048
D_MODEL = 2048
HEAD_DIM = 128
N_CORES = 8
ROPE_BASE = 10000.0

HPC = 2                      # heads per core
F_LOC = HPC * HEAD_DIM       # 256 local projection features per core
TCH = 512                    # token chunk width (stages 1/3)
QCH = 512                    # query chunk width (stage 2)
SCALE = 1.0 / math.sqrt(HEAD_DIM)
S_LOOK = 3                   # score-matmul lookahead in the attention loop

NPBF16 = ml_dtypes.bfloat16


def build_module(b=B, t=T, d_model=D_MODEL):
    bt = b * t
    dt_ = d_model // 128     # 16 contraction tiles
    kt = t // 128            # 16 key tiles per (batch, head)
    tch = TCH
    qch = QCH
    nqc = t // qch           # 4 query chunks
    cpb = t // tch           # 4 stage-1/3 token chunks per batch

    nc = bacc.Bacc(None, target_bir_lowering=False)

    xT = nc.dram_tensor("xT", [d_model, bt], BF16, kind="ExternalInput")
    wqT = nc.dram_tensor("wqT", [d_model, F_LOC], BF16, kind="ExternalInput")
    wkT = nc.dram_tensor("wkT", [d_model, F_LOC], BF16, kind="ExternalInput")
    wvT = nc.dram_tensor("wvT", [d_model, F_LOC], BF16, kind="ExternalInput")
    woT = nc.dram_tensor("woT", [F_LOC, d_model], BF16, kind="ExternalInput")
    cosT = nc.dram_tensor("cosT", [HEAD_DIM, t], F32, kind="ExternalInput")
    rsinT = nc.dram_tensor("rsinT", [HEAD_DIM, t], F32, kind="ExternalInput")
    ident = nc.dram_tensor("ident", [128, 128], F32, kind="ExternalInput")
    onesd = nc.dram_tensor("onesd", [128, 1], F16, kind="ExternalInput")
    onesb = nc.dram_tensor("onesb", [1, 128], F32, kind="ExternalInput")
    outP = nc.dram_tensor("outP", [d_model, bt], BF16, kind="ExternalOutput")

    with tile.TileContext(nc) as tc:
        with (
            tc.tile_pool(name="const", bufs=1) as constp,
            tc.tile_pool(name="wpool", bufs=1) as wpool,
            tc.tile_pool(name="xpool", bufs=3) as xpool,
            tc.tile_pool(name="qkv", bufs=2) as qkvp,
            tc.tile_pool(name="rope", bufs=2) as ropep,
            tc.tile_pool(name="vstg", bufs=2) as vstg,
            tc.tile_pool(name="epool", bufs=4) as epool,
            tc.tile_pool(name="attn", bufs=2) as attnp,
            tc.tile_pool(name="acc", bufs=2) as accp,
            tc.tile_pool(name="nrm", bufs=1) as nrmp,
            tc.tile_pool(name="opool", bufs=3) as opool,
            tc.tile_pool(name="ps_sp", bufs=2, space="PSUM") as ps_sp,
            tc.tile_pool(name="ps_pv", bufs=3, space="PSUM") as ps_pv,
            tc.tile_pool(name="ps_bc", bufs=1, space="PSUM") as ps_bc,
        ):
            # ---- constants + weights ----
            # sync ring: wq + first x chunk interleaved (gates first matmuls)
            # scalar ring: wk, wv (needed a few us in), wo later
            # gpsimd ring: cos/rsin (rope, ~4us in), identity/ones
            wq_sb = wpool.tile([128, dt_, F_LOC], BF16, tag="wq")
            wk_sb = wpool.tile([128, dt_, F_LOC], BF16, tag="wk")
            wv_sb = wpool.tile([128, dt_, F_LOC], BF16, tag="wv")
            x_sb0 = [
                xpool.tile([128, dt_ // 2, tch], BF16, tag="x", name=f"x0_{i}")
                for i in range(2)
            ]
            xsrc0 = xT[:, 0:tch].rearrange("(dt p) tt -> p dt tt", p=128)
            qtr = dt_ // 4
            for q4 in range(4):
                dsl = slice(q4 * qtr, (q4 + 1) * qtr)
                nc.sync.dma_start(
                    out=wq_sb[:, dsl, :],
                    in_=wqT[:, :].rearrange("(dt p) f -> p dt f", p=128)[:, dsl, :],
                )
            for q4 in range(4):
                dsl = slice(q4 * qtr, (q4 + 1) * qtr)
                lsl4 = slice((q4 % 2) * qtr, (q4 % 2 + 1) * qtr)
                nc.sync.dma_start(
                    out=x_sb0[q4 // 2][:, lsl4, :], in_=xsrc0[:, dsl, :]
                )
            nc.scalar.dma_start(
                out=wk_sb, in_=wkT[:, :].rearrange("(dt p) f -> p dt f", p=128)
            )
            nc.scalar.dma_start(
                out=wv_sb, in_=wvT[:, :].rearrange("(dt p) f -> p dt f", p=128)
            )
            cos_sb = constp.tile([128, t], F32)
            nc.gpsimd.dma_start(out=cos_sb, in_=cosT[:, :])
            rsin_sb = constp.tile([128, t], F32)
            nc.gpsimd.dma_start(out=rsin_sb, in_=rsinT[:, :])
            id_sb = constp.tile([128, 128], F32)
            nc.gpsimd.dma_start(out=id_sb, in_=ident[:, :])
            onesd_sb = constp.tile([128, 1], F16)
            nc.gpsimd.dma_start(out=onesd_sb, in_=onesd[:, :])
            onesb_sb = constp.tile([1, 128], F32R)
            nc.gpsimd.dma_start(out=onesb_sb, in_=onesb[:, :].bitcast(F32R))
            wo_sb = wpool.tile([128, HPC, d_model], BF16, tag="wo")
            nc.scalar.dma_start(
                out=wo_sb, in_=woT[:, :].rearrange("(ft p) d -> p ft d", p=128)
            )
            w_sbs = [wq_sb, wk_sb, wv_sb]
            dh = dt_ // 2

            def emit_s1(bi):
                """Projections + rope + v-transpose for batch bi.
                Returns (q_sb, k_sb, v_sb) bf16 SBUF tiles. Each (pi, chunk)
                uses one 2-bank psum pair: ft=0 in the low half, ft=1 high."""
                q_sb = qkvp.tile([128, HPC, t], BF16, tag="q")
                k_sb = qkvp.tile([128, HPC, t], BF16, tag="k")
                v_sb = qkvp.tile([128, HPC, kt, 128], F16, tag="v")
                qk_dst = [q_sb, k_sb]
                for ci in range(cpb):
                    off = ci * tch
                    lsl = slice(off, off + tch)
                    tsl = slice(bi * t + off, bi * t + off + tch)
                    if bi == 0 and ci == 0:
                        xh = x_sb0
                    else:
                        xh = []
                        for hi in range(2):
                            xt_ = xpool.tile([128, dh, tch], BF16, tag="x")
                            nc.sync.dma_start(
                                out=xt_,
                                in_=xT[:, tsl].rearrange(
                                    "(dt p) tt -> p dt tt", p=128
                                )[:, hi * dh : (hi + 1) * dh, :],
                            )
                            xh.append(xt_)
                    for pi in range(3):
                        pr = ps_sp.tile([128, 2 * tch], F32, tag="sp")
                        for ft in range(HPC):
                            hsl = slice(ft * tch, (ft + 1) * tch)
                            for di in range(dt_):
                                nc.tensor.matmul(
                                    pr[:, hsl],
                                    w_sbs[pi][:, di, ft * 128 : (ft + 1) * 128],
                                    xh[di // dh][:, di % dh, :],
                                    start=(di == 0),
                                    stop=(di == dt_ - 1),
                                )
                        if pi < 2:
                            # rope: out = in*cos + rot_half(in)*rsin
                            for ft in range(HPC):
                                hsl = slice(ft * tch, (ft + 1) * tch)
                                ro = ropep.tile([128, tch], F32, tag="ro")
                                nc.vector.tensor_mul(
                                    ro, pr[:, hsl], cos_sb[:, lsl]
                                )
                                rt = ropep.tile([128, tch], F32, tag="rt")
                                nc.vector.tensor_mul(
                                    rt[0:64], pr[64:128, hsl], rsin_sb[0:64, lsl]
                                )
                                nc.vector.tensor_mul(
                                    rt[64:128], pr[0:64, hsl],
                                    rsin_sb[64:128, lsl],
                                )
                                nc.vector.tensor_add(
                                    qk_dst[pi][:, ft, lsl], ro, rt
                                )
                        else:
                            vsb = vstg.tile([128, 2 * tch], F32, tag="vs")
                            nc.scalar.activation(vsb, pr, COPY)
                            for ft in range(HPC):
                                pst = ps_pv.tile([128, tch], F32, tag="pv")
                                for j in range(tch // 128):
                                    js = ft * tch + j * 128
                                    nc.tensor.transpose(
                                        pst[:, j * 128 : (j + 1) * 128],
                                        vsb[:, js : js + 128],
                                        id_sb,
                                    )
                                nc.vector.tensor_copy(
                                    v_sb[:, ft, ci * 4 : (ci + 1) * 4, :]
                                    .rearrange("p a b -> p (a b)"),
                                    pst,
                                )
                return q_sb, k_sb, v_sb

            def emit_s2(bi, h, qkv, attn_n):
                """Attention for (batch bi, local head h) -> writes
                attn_n[:, h, :] (normalized, bf16). Two kti sweeps, each
                covering a 1024-wide q window: paired score psums (one exp
                per pair, k-stationary shared), pv per 512 half with the
                v-stationary shared, softmax denominator accumulated
                elementwise on DVE (low half) + gpsimd (high half) with a
                bf16 cast on the last add, then a rank-1 ones matmul per
                512-q chunk reduces over the 128 k-partitions."""
                q_sb, k_sb, v_sb = qkv
                attn_u = attnp.tile([128, t], F32, tag="au")
                den_row = nrmp.tile([1, t], F32, tag="dr")
                den_t = nrmp.tile([128, t // 128], F32, tag="dT")
                rec_t = nrmp.tile([128, t // 128], F32, tag="rT")
                rec_row = nrmp.tile([1, t], F32R, tag="rr")
                pending = [None]

                def flush_reduce():
                    if pending[0] is not None:
                        pending[0]()
                        pending[0] = None

                for qp in range(t // (2 * qch)):
                    q0 = slice(qp * 2 * qch, qp * 2 * qch + qch)
                    q1 = slice(qp * 2 * qch + qch, (qp + 1) * 2 * qch)
                    e_tiles = [None] * kt

                    def emit_score(kti):
                        sps = ps_sp.tile([128, 2 * qch], F32, tag="sp")
                        ktile = k_sb[:, h, kti * 128 : (kti + 1) * 128]
                        nc.tensor.matmul(
                            sps[:, :qch], ktile, q_sb[:, h, q0],
                            start=True, stop=True,
                        )
                        nc.tensor.matmul(
                            sps[:, qch:], ktile, q_sb[:, h, q1],
                            start=True, stop=True,
                        )
                        e_sb = epool.tile([128, 2 * qch], F16, tag="E")
                        nc.scalar.activation(e_sb, sps, EXP, scale=SCALE)
                        e_tiles[kti] = e_sb

                    for kti in range(2):
                        emit_score(kti)
                    # previous sweep's denominator reduce lands here so the
                    # PE has the new sweep's scores queued ahead of it
                    flush_reduce()
                    pv0 = ps_pv.tile([128, qch], F32, tag="pv")
                    pv1 = ps_pv.tile([128, qch], F32, tag="pv")
                    accD = accG = abD = abG = None
                    for kti in range(kt):
                        e = e_tiles[kti]
                        vtile = v_sb[:, h, kti, :]
                        nc.tensor.matmul(
                            pv0, vtile, e[:, :qch],
                            start=(kti == 0), stop=(kti == kt - 1),
                        )
                        nc.tensor.matmul(
                            pv1, vtile, e[:, qch:],
                            start=(kti == 0), stop=(kti == kt - 1),
                        )
                        if kti + 2 < kt:
                            emit_score(kti + 2)
                        if kti == 1:
                            accD = accp.tile([128, qch], F16, tag="aD")
                            nc.vector.tensor_add(
                                accD, e_tiles[0][:, :qch], e[:, :qch]
                            )
                            accG = accp.tile([128, qch], F16, tag="aG")
                            nc.vector.tensor_add(
                                accG, e_tiles[0][:, qch:], e[:, qch:]
                            )
                        elif kti > 1 and kti < kt - 1:
                            nD = accp.tile([128, qch], F16, tag="aD")
                            nc.vector.tensor_add(nD, accD, e[:, :qch])
                            accD = nD
                            nG = accp.tile([128, qch], F16, tag="aG")
                            nc.vector.tensor_add(nG, accG, e[:, qch:])
                            accG = nG
                    # evacuate the pv psums first (frees the pv ring for the
                    # next sweep), then the final den adds
                    nc.vector.tensor_copy(attn_u[:, q0], pv0)
                    nc.vector.tensor_copy(attn_u[:, q1], pv1)
                    e15 = e_tiles[kt - 1]
                    abD = accp.tile([128, qch], F16, tag="bD")
                    nc.vector.tensor_add(abD, accD, e15[:, :qch])
                    abG = accp.tile([128, qch], F16, tag="bG")
                    nc.vector.tensor_add(abG, accG, e15[:, qch:])

                    def make_reduce(abD_=abD, abG_=abG, q0_=q0, q1_=q1):
                        def red():
                            # reduce over k-partitions + stage the den row
                            for ab, qs in ((abD_, q0_), (abG_, q1_)):
                                dn = ps_bc.tile([128, qch], F32, tag="bc")
                                nc.tensor.matmul(
                                    dn[0:1, :], onesd_sb, ab,
                                    start=True, stop=True,
                                )
                                nc.vector.tensor_copy(
                                    den_row[:, qs], dn[0:1, :]
                                )
                        return red

                    pending[0] = make_reduce()
                flush_reduce()
                # re-partition the [1,t] denominator row to [128, t/128]
                # (element m of the row -> (m // 16, m % 16)), reciprocal in
                # the parallel layout, scatter back with the inverse mapping.
                nc.gpsimd.dma_start(out=den_t, in_=den_row)
                nc.vector.reciprocal(rec_t, den_t)
                nc.gpsimd.dma_start(
                    out=rec_row, in_=rec_t[:, :].bitcast(F32R)
                )

                def finish():
                    # deferred: emitted after the next block so the PE never
                    # drains waiting on the reciprocal round-trip
                    for qc in range(nqc):
                        qsl = slice(qc * qch, (qc + 1) * qch)
                        rbc = ps_bc.tile([128, qch], F32, tag="bc")
                        nc.tensor.matmul(
                            rbc,
                            onesb_sb,
                            rec_row[:, qsl],
                            start=True,
                            stop=True,
                        )
                        nc.vector.tensor_mul(
                            attn_n[:, h, qsl], attn_u[:, qsl], rbc
                        )

                return finish

            def emit_s3(bi, attn_n):
                """Fused out-projection for batch bi (partial product over
                this core's 256 features, full d_model columns). Two d-tiles
                per 2-bank psum pair; one wide evacuation + DMA per pair."""
                for c4 in range(cpb):
                    off = c4 * tch
                    gsl = slice(bi * t + off, bi * t + off + tch)
                    for dp in range(dt_ // 2):
                        pr = ps_sp.tile([128, 2 * tch], F32, tag="sp")
                        for half in range(2):
                            do = 2 * dp + half
                            hsl = slice(half * tch, (half + 1) * tch)
                            for ft in range(HPC):
                                nc.tensor.matmul(
                                    pr[:, hsl],
                                    wo_sb[:, ft, do * 128 : (do + 1) * 128],
                                    attn_n[:, ft, off : off + tch],
                                    start=(ft == 0),
                                    stop=(ft == HPC - 1),
                                )
                        osb = opool.tile([128, 2 * tch], BF16, tag="o")
                        dst = outP[
                            2 * dp * 128 : (2 * dp + 2) * 128, gsl
                        ].rearrange("(j p) c -> p j c", p=128)
                        src = osb[:, :].rearrange("p (j c) -> p j c", j=2)
                        if dp % 2 == 0:
                            nc.vector.tensor_copy(osb, pr)
                            nc.gpsimd.dma_start(out=dst, in_=src)
                        else:
                            nc.scalar.activation(osb, pr, COPY)
                            nc.sync.dma_start(out=dst, in_=src)

            # ---- interleaved schedule: PE never drains at batch edges ----
            qkv = emit_s1(0)
            for bi in range(b):
                attn_n = attnp.tile([128, HPC, t], BF16, tag="an")
                f0 = emit_s2(bi, 0, qkv, attn_n)
                f1 = emit_s2(bi, 1, qkv, attn_n)
                f0()
                if bi + 1 < b:
                    qkv = emit_s1(bi + 1)
                f1()
                emit_s3(bi, attn_n)

    nc.finalize()
    return nc


_module_cache = {}


def _get_module(b, t, d_model):
    key = (b, t, d_model)
    if key not in _module_cache:
        _module_cache[key] = build_module(b, t, d_model)
    return _module_cache[key]


def _host_tables(t):
    half = HEAD_DIM // 2
    theta = 1.0 / (
        np.float32(ROPE_BASE)
        ** (np.arange(half, dtype=np.float32) / np.float32(half))
    )
    freqs = np.arange(t, dtype=np.float32)[:, None] * theta[None, :]
    emb = np.concatenate([freqs, freqs], axis=-1)  # (t, 128)
    cosT = np.ascontiguousarray(np.cos(emb).T.astype(np.float32))
    sinT = np.sin(emb).T.astype(np.float32)
    rsinT = sinT.copy()
    rsinT[:half] = -sinT[:half]
    rsinT = np.ascontiguousarray(rsinT)
    return cosT, rsinT


def _run(x, Wq, Wk, Wv, Wo, trace=False):
    b_, t_, d_ = x.shape
    n_cores = (d_ // HEAD_DIM) // HPC
    nc = _get_module(b_, t_, d_)

    xT = np.ascontiguousarray(
        x.reshape(b_ * t_, d_).T.astype(NPBF16)
    )
    cosT, rsinT = _host_tables(t_)
    ident = np.eye(128, dtype=np.float32)
    onesd = np.ones((128, 1), dtype=np.float16)
    onesb = np.ones((1, 128), dtype=np.float32)

    in_maps = []
    for c in range(n_cores):
        fs = slice(c * F_LOC, (c + 1) * F_LOC)
        in_maps.append(
            {
                "xT": xT,
                "wqT": np.ascontiguousarray(Wq[fs, :].T.astype(NPBF16)),
                "wkT": np.ascontiguousarray(Wk[fs, :].T.astype(NPBF16)),
                "wvT": np.ascontiguousarray(Wv[fs, :].T.astype(NPBF16)),
                "woT": np.ascontiguousarray(Wo[:, fs].T.astype(NPBF16)),
                "cosT": cosT,
                "rsinT": rsinT,
                "ident": ident,
                "onesd": onesd,
                "onesb": onesb,
            }
        )
    res = run_bass_kernel_spmd(
        nc, in_maps, core_ids=list(range(n_cores)), trace=trace
    )
    acc = res.results[0]["outP"].astype(np.float32)
    for c in range(1, n_cores):
        acc += res.results[c]["outP"].astype(np.float32)
    out = np.ascontiguousarray(acc.T).reshape(b_, t_, d_)
    return out, res


def kernel(x, Wq, Wk, Wv, Wo):
    x = np.asarray(x, dtype=np.float32)
    Wq = np.asarray(Wq, dtype=np.float32)
    Wk = np.asarray(Wk, dtype=np.float32)
    Wv = np.asarray(Wv, dtype=np.float32)
    Wo = np.asarray(Wo, dtype=np.float32)
    out, _ = _run(x, Wq, Wk, Wv, Wo, trace=False)
    return out
